# revision 1
# baseline (speedup 1.0000x reference)
"""MiniBatchDiscrimination Trainium2 kernel (8-core SPMD).

Reference computation:
    m = (x @ T).reshape(B, OUT_F, NUM_K)            # B=256, OUT_F=128, NUM_K=16
    dists = |m[None,:,:,:] - m[:,None,:,:]|         # [B, B, OUT_F, NUM_K]
    out = sum_i exp(-sum_k dists) - 1               # [B, OUT_F]
    return concat([x, out], axis=-1)                # [B, 640]

Strategy (per core, identical SPMD program, per-core data):
  * Each core owns JB=32 output rows (j). Full m is computed on every core
    (replicated GEMM, cheap) so no collectives are needed.
  * m is stored in SBUF as [partition p=(f8,k), free n=(i,f_o)] with
    f = f_o*8 + f8, p = f8*16 + k. Then T's columns c = f*16+k satisfy
    c = f_o*128 + p, i.e. each f_o corresponds to a contiguous 128-column
    block of T -> the GEMM producing this layout is 16 plain matmuls.
  * Pairwise pass per i: DVE bf16 tensor_sub (2x mode) of m_shard vs
    m_all[:,i] broadcast over j; abs on ACT (Abs activation) or DVE
    (tensor_scalar abs_max, 4x mode) -- split tunable; then the k-sum is a
    TensorE matmul with a block-diagonal ones [128,8] writing dist rows for
    16 consecutive i into one PSUM bank [128=(i_sub,f8), 512=(j,f_o)].
  * exp(-dist): ACT Exp with scale=-1 from PSUM -> bf16 SBUF.
  * sum over i: TensorE matmuls with a second ones pattern accumulating in
    PSUM across all 16 blocks -> [8, 512] = out[f8, (j, f_o)].
  * Host unshards: reshape to [32,128] per core, concat with x.
"""

import os
import numpy as np

import concourse.bass as bass
import concourse.tile as tile
from concourse import bacc, mybir

BF16 = mybir.dt.bfloat16
FP32 = mybir.dt.float32
NPBF16 = np.dtype(mybir.dt.np(BF16))

B = 256
IN_F = 512
OUT_F = 128
NUM_K = 16
N_CORES = 8
JB = B // N_CORES          # 32 j-rows owned per core
F8 = 8                     # f8 = f % 8   (partition group)
FO = OUT_F // F8           # 16 f_o values (free dim)
KC = IN_F // 128           # 4 contraction chunks for the GEMM
NBLK = B // 16             # 16 i-blocks of 16
SUBI = 8                   # i's per DVE sub instruction
# per 8-i sub-batch, how many i's of the abs go to the scalar engine
# (the rest run on DVE as 4x-mode sign-bit stripping)
ACT_ABS_N = int(os.environ.get("ACT_ABS_N", "5"))


def build_nc():
    nc = bacc.Bacc(name="minibatch_discrim")

    # host-prearranged [p, c, i] so each partition's DMA line is contiguous;
    # columns B..B+JB repeat this core's own j-columns so one FD=288 matmul
    # produces m_all and m_sh together (identical values -> exact diagonal).
    xT_d = nc.dram_tensor("xT", [128, KC, B + JB], BF16, kind="ExternalInput")
    # T pre-permuted on host to [fo][p][c][128 cols] so each fo block is
    # one contiguous 128KB DMA (1KB per partition line) that unblocks that
    # fo's GEMM immediately.
    T_d = nc.dram_tensor("T_w", [FO, 128, KC, 128], BF16, kind="ExternalInput")
    onk_d = nc.dram_tensor("ones_k", [128, 8 * 64], BF16, kind="ExternalInput")
    ona_d = nc.dram_tensor("ones_acc", [128, F8], BF16, kind="ExternalInput")
    out_d = nc.dram_tensor("out_pair", [F8, JB * FO], FP32, kind="ExternalOutput")

    with tile.TileContext(nc) as tc:
        with (
            tc.tile_pool(name="const", bufs=1) as constp,
            tc.tile_pool(name="mm", bufs=1) as mmp,
            tc.tile_pool(name="gpsum", bufs=4, space=bass.MemorySpace.PSUM) as gps,
            tc.tile_pool(name="dpsum", bufs=3, space=bass.MemorySpace.PSUM) as dps,
            tc.tile_pool(name="apsum", bufs=1, space=bass.MemorySpace.PSUM) as aps,
            tc.tile_pool(name="work", bufs=3) as wp,
            tc.tile_pool(name="expp", bufs=3) as ep,
        ):
            # ---- constants / inputs to SBUF ----
            zero_b = constp.tile([128, 1], FP32)
            nc.gpsimd.memset(zero_b[:], 0.0)
            neg1_b = constp.tile([128, 1], FP32)
            nc.gpsimd.memset(neg1_b[:], -1.0)

            # ones_k[:, q8, (q, f8)] = 1 iff q == q8 and p//16 == f8.
            # The k-reduce matmul for i_sub targets the 64-partition slice at
            # offset (isub//8)*64 using pattern q8 = isub%8: its 8 target rows
            # get sum_k, the other 56 rows of the slice accumulate += 0.
            ones_k = constp.tile([128, 8, 64], BF16)
            nc.sync.dma_start(ones_k[:], onk_d.rearrange("p (s q) -> p s q", q=64))
            ones_a = constp.tile([128, F8], BF16)
            nc.sync.dma_start(ones_a[:], ona_d[:])

            # warm the ACT exp/abs table while DMAs run
            warm = constp.tile([128, 1], FP32)
            nc.scalar.activation(
                warm[:], zero_b[:], mybir.ActivationFunctionType.Exp, bias=zero_b[:]
            )

            # xT as [p, c, i]  (contraction chunk c)
            xT_sb = constp.tile([128, KC, B + JB], BF16)
            nc.sync.dma_start(xT_sb[:], xT_d[:])
            # T per-fo tiles; 16 independent DMAs so fo-GEMMs start as soon
            # as their slice lands.
            T_tiles = []
            for fo in range(FO):
                tt = constp.tile([128, KC, 128], BF16, tag=f"T{fo}")
                nc.sync.dma_start(tt[:], T_d[fo])
                T_tiles.append(tt)

            # ---- GEMM: m_all [p=(f8,k), (i, f_o)], m_sh [p, (j, f_o)] ----
            m_all = mmp.tile([128, B, FO], BF16)
            m_sh = mmp.tile([128, JB, FO], BF16)
            for fo in range(FO):
                pm = gps.tile([128, B + JB], FP32, tag="gemm_full")
                for c in range(KC):
                    nc.tensor.matmul(
                        pm[:],
                        T_tiles[fo][:, c, :],
                        xT_sb[:, c, :],
                        start=(c == 0),
                        stop=(c == KC - 1),
                    )
                nc.scalar.copy(m_all[:, :, fo], pm[:, :B])
                nc.vector.tensor_copy(m_sh[:, :, fo], pm[:, B:])

            # ---- main pairwise loop ----
            acc = aps.tile([F8, JB * FO], FP32)  # sum over i of exp(-dist)
            sub_idx = 0
            for blk in range(NBLK):
                pd = dps.tile([128, JB * FO], FP32, tag="dist")
                for h in range(16 // SUBI):
                    i0 = blk * 16 + h * SUBI
                    diff = wp.tile([128, SUBI, JB, FO], BF16, tag="diff")
                    nc.vector.tensor_sub(
                        diff[:],
                        m_sh[:, None, :, :].broadcast_to([128, SUBI, JB, FO]),
                        m_all[:, i0:i0 + SUBI, None, :].broadcast_to(
                            [128, SUBI, JB, FO]
                        ),
                    )
                    ad = wp.tile([128, SUBI, JB, FO], BF16, tag="absd")
                    na = ACT_ABS_N
                    if na > 0:
                        nc.scalar.activation(
                            ad[:, :na], diff[:, :na],
                            mybir.ActivationFunctionType.Abs, bias=zero_b[:],
                        )
                    if na < SUBI:
                        # |x| on DVE at 4x mode: strip the bf16 sign bit
                        nc.vector.tensor_scalar(
                            ad[:, na:].bitcast(mybir.dt.uint16),
                            diff[:, na:].bitcast(mybir.dt.uint16),
                            0x7FFF, None, op0=mybir.AluOpType.bitwise_and,
                        )
                    sub_idx += 1
                    for s in range(SUBI):
                        isub = h * SUBI + s
                        g, q = isub // 8, isub % 8
                        nc.tensor.matmul(
                            pd[g * 64:(g + 1) * 64, :],
                            ones_k[:, q, :],
                            ad[:, s, :, :],
                            start=(q == 0),
                            stop=(q == 7),
                        )
                et = ep.tile([128, JB * FO], BF16, tag="expt")
                nc.scalar.activation(
                    et[:], pd[:],
                    mybir.ActivationFunctionType.Exp, bias=zero_b[:], scale=-1.0,
                )
                nc.tensor.matmul(
                    acc[:],
                    ones_a[:],
                    et[:],
                    start=(blk == 0),
                    stop=(blk == NBLK - 1),
                    skip_group_check=True,
                )

            # ---- tail: subtract 1, store ----
            fin = mmp.tile([F8, JB * FO], FP32)
            nc.vector.tensor_scalar_add(fin[:], acc[:], -1.0)
            nc.sync.dma_start(out_d[:], fin[:])

    nc.finalize()
    return nc


def make_in_maps(x: np.ndarray, T: np.ndarray):
    # xT_h[p, c, i] = x[i, c*128+p]
    xT_h = np.ascontiguousarray(
        x.T.astype(NPBF16).reshape(KC, 128, B).transpose(1, 0, 2)
    )
    T_b = np.ascontiguousarray(T).astype(NPBF16)           # [512, 2048]

    p = np.arange(128)[:, None]
    r = np.arange(F8)[None, :]
    ones_a = np.ascontiguousarray((p % 8 == r).astype(NPBF16))    # [128,8]
    # ones_k[p, q8, q] = 1 iff q == q8*8 + p//16  (q in 0..63)
    q = np.arange(64)[None, None, :]
    s = np.arange(8)[None, :, None]
    ones_k = (q == s * 8 + p[:, :, None] // 16).astype(NPBF16)
    ones_k = np.ascontiguousarray(ones_k.reshape(128, 8 * 64))

    # T_w host-permuted to [fo, p, c, n]: T_perm[fo, p, c, n] = T[c*128+p, fo*128+n]
    T_perm = np.ascontiguousarray(
        T_b.reshape(KC, 128, FO, 128).transpose(2, 1, 0, 3)
    )

    in_maps = []
    for c in range(N_CORES):
        xTc = np.ascontiguousarray(np.concatenate(
            [xT_h, xT_h[:, :, c * JB:(c + 1) * JB]], axis=2
        ))
        in_maps.append({
            "xT": xTc,
            "T_w": T_perm,
            "ones_k": ones_k,
            "ones_acc": ones_a,
        })
    return in_maps


def assemble(x: np.ndarray, pair_parts) -> np.ndarray:
    """pair_parts: list of [8, JB*FO] fp32 per core -> full [B, IN_F+OUT_F]."""
    out = np.empty((B, IN_F + OUT_F), np.float32)
    out[:, :IN_F] = x
    for c, fp in enumerate(pair_parts):
        # fp[f8, j*FO + fo] -> out[c*JB + j, IN_F + fo*8 + f8]
        blk = fp.reshape(F8, JB, FO).transpose(1, 2, 0).reshape(JB, OUT_F)
        out[c * JB:(c + 1) * JB, IN_F:] = blk
    return out


_NC_CACHE = None


def kernel(x: np.ndarray, T: np.ndarray) -> np.ndarray:
    global _NC_CACHE
    from concourse import bass_utils

    if _NC_CACHE is None:
        _NC_CACHE = build_nc()
    nc = _NC_CACHE
    in_maps = make_in_maps(np.asarray(x, np.float32), np.asarray(T, np.float32))
    res = bass_utils.run_bass_kernel_spmd(nc, in_maps, core_ids=list(range(N_CORES)))
    parts = [r["out_pair"].astype(np.float32) for r in res.results]
    return assemble(np.asarray(x, np.float32), parts)



# revision 10
# speedup vs baseline: 1.3474x; 1.3474x over previous
"""MiniBatchDiscrimination Trainium2 kernel (8-core SPMD, circulant strips).

Reference computation:
    m = (x @ T).reshape(B, OUT_F, NUM_K)            # B=256, OUT_F=128, NUM_K=16
    dists = |m[None,:,:,:] - m[:,None,:,:]|         # [B, B, OUT_F, NUM_K]
    out = sum_i exp(-sum_k dists) - 1               # [B, OUT_F]
    return concat([x, out], axis=-1)                # [B, 640]

Strategy (identical SPMD program; per-core data = a column permutation):
  * The BxB pair matrix is covered once per unordered pair by 16 "strips":
    strip a = {i in 16-block a} x {j in 16-blocks a..a+8 (mod 16)}.  Each
    strip contributes row-sums for its j's (partial_b, sum over i) and,
    for the inner window blocks a+1..a+7 only, row-sums for its i's
    (partial_a, sum over j).  Exact cover: source-block offset e=(bj-bi)%16
    is counted by partial_b iff e in {0} u [8,15] and by partial_a iff
    e in [1,7].  Host accumulates partials from all cores and subtracts 1.
  * Core c owns strips 2c and 2c+1.  Their window union is 10 consecutive
    16-blocks -> 160 "virtual" columns; the host permutes x's rows per core
    so the program is core-independent (SPMD with full input replication
    of T; x columns gathered per core).
  * Per core work: GEMM m2[p=(f8,k), vcol, fo] (bf16, fo innermost so the
    pairwise subs run in DVE 2x mode), then per (strip, j-chunk of 32):
    tensor_sub (DVE 2x) -> |.| (split ACT Abs / DVE 4x sign-strip / GPSIMD
    sign-strip) -> k-sum on TensorE (block-diagonal ones, FD=512) -> Exp on
    ACT -> partial_b ones-matmul into packed PSUM stripes; partial_a via a
    small add-tree (GPSIMD) into SBUF slots.
"""

import os
import numpy as np

import concourse.bass as bass
import concourse.tile as tile
from concourse import bacc, mybir

BF16 = mybir.dt.bfloat16
FP32 = mybir.dt.float32
U16 = mybir.dt.uint16
NPBF16 = np.dtype(mybir.dt.np(BF16))

B = 256
IN_F = 512
OUT_F = 128
NUM_K = 16
N_CORES = 8
F8 = 8
FO = OUT_F // F8           # 16 fo groups (free dim)
KC = IN_F // 128           # 4 contraction chunks
NVB = 10                   # virtual 16-blocks per core
VCOLS = NVB * 16           # 160
NST = 2                    # strips per core
CH = [(0, 32), (32, 32), (64, 32), (96, 32), (128, 16)]  # (window j-off, len)

# abs engine per unit (st-major, then chunk, then i-half): A=ACT, G=GPSIMD,
# D=DVE.  Units 8,9,18,19 are the 16-j tail chunks.
_DEF = "AADAADADDA" "AADAADADAA"
ABS_SCHED = os.environ.get("ABS_SCHED", _DEF)
# engine for the partial_a add-tree: G (gpsimd) or D (DVE)
TREE_ENG = os.environ.get("TREE_ENG", "G")


def build_nc():
    nc = bacc.Bacc(name="mbd_strips")

    xT_d = nc.dram_tensor("xT", [128, KC, VCOLS], BF16, kind="ExternalInput")
    T_d = nc.dram_tensor("T_w", [FO, 128, KC, 128], BF16, kind="ExternalInput")
    onk_d = nc.dram_tensor("ones_k", [128, 8 * 64], BF16, kind="ExternalInput")
    ona_d = nc.dram_tensor("ones_acc", [128, F8], BF16, kind="ExternalInput")
    accB_d = nc.dram_tensor("accB", [128, 4, 512], FP32, kind="ExternalOutput")
    accA_d = nc.dram_tensor("accA", [128, NST, 4, FO], FP32, kind="ExternalOutput")

    with tile.TileContext(nc) as tc:
        with (
            tc.tile_pool(name="const", bufs=1) as constp,
            tc.tile_pool(name="gpsum", bufs=2, space=bass.MemorySpace.PSUM) as gps,
            tc.tile_pool(name="dpsum", bufs=2, space=bass.MemorySpace.PSUM) as dps,
            tc.tile_pool(name="bpsum", bufs=1, space=bass.MemorySpace.PSUM) as bps,
            tc.tile_pool(name="diffp", bufs=3) as dfp,
            tc.tile_pool(name="adp", bufs=3) as adp,
            tc.tile_pool(name="expp", bufs=3) as ep,
            tc.tile_pool(name="treep", bufs=2) as tp,
            tc.tile_pool(name="outp", bufs=1) as op_,
        ):
            zero_b = constp.tile([128, 1], FP32)
            nc.gpsimd.memset(zero_b[:], 0.0)

            ones_k = constp.tile([128, 8, 64], BF16)
            nc.sync.dma_start(ones_k[:], onk_d.rearrange("p (s q) -> p s q", q=64))
            ones_a = constp.tile([128, F8], BF16)
            nc.sync.dma_start(ones_a[:], ona_d[:])

            # warm the ACT exp/abs tables while DMAs land
            warm = constp.tile([128, 1], FP32)
            nc.scalar.activation(
                warm[:], zero_b[:], mybir.ActivationFunctionType.Exp, bias=zero_b[:]
            )

            xT_sb = constp.tile([128, KC, VCOLS], BF16)
            nc.sync.dma_start(xT_sb[:], xT_d[:])
            T_tiles = []
            for fo in range(FO):
                tt = constp.tile([128, KC, 128], BF16, tag=f"T{fo}")
                nc.sync.dma_start(tt[:], T_d[fo])
                T_tiles.append(tt)

            # ---- GEMM: m2[p=(f8,k), vcol, fo] ----
            m2 = constp.tile([128, VCOLS, FO], BF16)
            for fo in range(FO):
                pm = gps.tile([128, VCOLS], FP32, tag="gemm")
                for c in range(KC):
                    nc.tensor.matmul(
                        pm[:],
                        T_tiles[fo][:, c, :],
                        xT_sb[:, c, :],
                        start=(c == 0),
                        stop=(c == KC - 1),
                    )
                nc.vector.tensor_copy(m2[:, :, fo], pm[:])

            # ---- persistent accumulators ----
            accB = []
            for b in range(4):
                accB_t = bps.tile([128, 512], FP32, tag=f"accB{b}", name=f"accB{b}")
                accB.append(accB_t)
            accA_sb = op_.tile([128, NST, 4, FO], FP32)

            u = 0
            for st in range(NST):
                ivc = st * 16
                for ci, (joff, jn) in enumerate(CH):
                    jvc = st * 16 + joff
                    fd = jn * FO
                    pd = dps.tile([128, 512], FP32, tag="dist")
                    for h in range(2):
                        i0 = ivc + h * 8
                        diff = dfp.tile([128, 8, 32, FO], BF16, tag="diff")
                        nc.vector.tensor_sub(
                            diff[:, :, :jn, :],
                            m2[:, i0:i0 + 8, None, :].broadcast_to(
                                [128, 8, jn, FO]
                            ),
                            m2[:, None, jvc:jvc + jn, :].broadcast_to(
                                [128, 8, jn, FO]
                            ),
                        )
                        ad = adp.tile([128, 8, 32, FO], BF16, tag="absd")
                        eng = ABS_SCHED[u]
                        if eng == "A":
                            nc.scalar.activation(
                                ad[:, :, :jn, :], diff[:, :, :jn, :],
                                mybir.ActivationFunctionType.Abs, bias=zero_b[:],
                            )
                        elif eng == "G":
                            nc.gpsimd.tensor_scalar(
                                ad[:, :, :jn, :], diff[:, :, :jn, :],
                                0.0, None, op0=mybir.AluOpType.abs_max,
                            )
                        else:
                            nc.vector.tensor_scalar(
                                ad[:, :, :jn, :].bitcast(U16),
                                diff[:, :, :jn, :].bitcast(U16),
                                0x7FFF, None, op0=mybir.AluOpType.bitwise_and,
                            )
                        u += 1
                        for s8 in range(8):
                            nc.tensor.matmul(
                                pd[h * 64:(h + 1) * 64, :fd],
                                ones_k[:, s8, :],
                                ad[:, s8, :jn, :],
                                start=(s8 == 0),
                                stop=(s8 == 7),
                            )
                    et = ep.tile([128, 512], BF16, tag="expt")
                    nc.scalar.activation(
                        et[:, :fd], pd[:, :fd],
                        mybir.ActivationFunctionType.Exp, bias=zero_b[:], scale=-1.0,
                    )
                    # partial_b -> packed psum stripe
                    sidx = st * 5 + ci
                    poff = 32 * (sidx % 3)
                    nc.tensor.matmul(
                        accB[sidx // 3][poff:poff + 8, :fd],
                        ones_a[:],
                        et[:, :fd],
                        start=True,
                        stop=True,
                        skip_group_check=True,
                    )
                    # partial_a add-tree over eligible j (window blocks st+1..st+7)
                    lo = max(16 - joff, 0)
                    hi = min(128 - joff, jn)
                    if lo < hi:
                        # flat 2D views: j-slice [lo,hi) = cols [lo*FO, hi*FO)
                        cur = et[:, lo * FO: hi * FO]
                        n = hi - lo
                        lvl = 0
                        tadd = (nc.gpsimd.tensor_add if TREE_ENG == "G"
                                else nc.vector.tensor_add)
                        while n > 1:
                            half = n // 2
                            dt = BF16 if lvl < 2 else FP32
                            if half == 1:
                                nxt = accA_sb[:, st, ci, :]
                            else:
                                ntile = tp.tile([128, half * FO], dt,
                                                tag=f"tr{lvl}",
                                                name=f"tr{st}_{ci}_{lvl}")
                                nxt = ntile[:]
                            tadd(nxt, cur[:, :half * FO],
                                 cur[:, half * FO: 2 * half * FO])
                            cur = nxt
                            n = half
                            lvl += 1

            # ---- tail: drain accB psum, store ----
            fin = op_.tile([128, 4, 512], FP32)
            for b in range(4):
                nc.scalar.copy(fin[:, b, :], accB[b][:])
            nc.sync.dma_start(accB_d[:], fin[:])
            nc.sync.dma_start(accA_d[:], accA_sb[:])

    nc.finalize()
    return nc


def _vcol_real(c):
    """virtual column -> real row index, for core c."""
    vb = np.arange(VCOLS) // 16
    s = np.arange(VCOLS) % 16
    return ((2 * c + vb) % 16) * 16 + s


def make_in_maps(x: np.ndarray, T: np.ndarray):
    # xT_h[p, ch, i] = x[i, ch*128+p]
    xT_h = np.ascontiguousarray(
        x.T.astype(NPBF16).reshape(KC, 128, B).transpose(1, 0, 2)
    )
    T_b = np.ascontiguousarray(T).astype(NPBF16)  # [512, 2048]

    p = np.arange(128)[:, None]
    r = np.arange(F8)[None, :]
    ones_a = np.ascontiguousarray((p % 8 == r).astype(NPBF16))  # [128,8]
    q = np.arange(64)[None, None, :]
    s = np.arange(8)[None, :, None]
    ones_k = (q == s * 8 + p[:, :, None] // 16).astype(NPBF16)
    ones_k = np.ascontiguousarray(ones_k.reshape(128, 8 * 64))

    # T_w[fo, p, c, n] = T[c*128+p, fo*128+n]
    T_perm = np.ascontiguousarray(
        T_b.reshape(KC, 128, FO, 128).transpose(2, 1, 0, 3)
    )

    in_maps = []
    for c in range(N_CORES):
        cols = _vcol_real(c)
        in_maps.append({
            "xT": np.ascontiguousarray(xT_h[:, :, cols]),
            "T_w": T_perm,
            "ones_k": ones_k,
            "ones_acc": ones_a,
        })
    return in_maps


def assemble(x: np.ndarray, results) -> np.ndarray:
    out_pair = np.zeros((B, OUT_F), np.float32)
    for c, res in enumerate(results):
        cols = _vcol_real(c)
        accB = res["accB"].astype(np.float32)        # [128, 4, 512]
        accA = res["accA"].astype(np.float32)        # [128, NST, 4, FO]
        for st in range(NST):
            for ci, (joff, jn) in enumerate(CH):
                sidx = st * 5 + ci
                poff = 32 * (sidx % 3)
                vals = accB[poff:poff + 8, sidx // 3, :jn * FO].reshape(8, jn, FO)
                # out[j, fo*8+f8] += vals[f8, jj, fo]
                rows = cols[st * 16 + joff: st * 16 + joff + jn]
                out_pair[rows] += vals.transpose(1, 2, 0).reshape(jn, OUT_F)
            ta = accA[:, st, :, :].sum(axis=1)       # [128, FO]
            # row p = s*8+f8 -> out[i(s), fo*8+f8]
            rows = cols[st * 16: st * 16 + 16]
            out_pair[rows] += ta.reshape(16, 8, FO).transpose(0, 2, 1).reshape(
                16, OUT_F)
    out_pair -= 1.0
    out = np.empty((B, IN_F + OUT_F), np.float32)
    out[:, :IN_F] = x
    out[:, IN_F:] = out_pair
    return out


_NC_CACHE = None


def kernel(x: np.ndarray, T: np.ndarray) -> np.ndarray:
    global _NC_CACHE
    from concourse import bass_utils

    if _NC_CACHE is None:
        _NC_CACHE = build_nc()
    nc = _NC_CACHE
    in_maps = make_in_maps(np.asarray(x, np.float32), np.asarray(T, np.float32))
    res = bass_utils.run_bass_kernel_spmd(nc, in_maps, core_ids=list(range(N_CORES)))
    return assemble(np.asarray(x, np.float32), res.results)
